# revision 1
# baseline (speedup 1.0000x reference)
"""Multi-head attention (b=8, n=1024, dim=1024, heads=16) on 8 Trainium2 cores.

v3: bf16 operands everywhere (inputs fed as bf16; psum/output stay fp32),
1024-column matmul streams (half the PE instruction count of v2), split
softmax normalization (pairs 0-3 normalized mid-flight, only pair 7's chain
is tail-serial), software-pipelined attention keeping the PE continuously
busy at full clock.

Per-core structure (core b computes head-attention for batch row b):
  V proj:    8 x [128,1024] psum tiles, 8 matmuls each (k), one strided DVE
             copy into v_aug (64 v-cols + ones column per head).
  attention: per head pair p, 16 steps (head-half hf, key-tile jt). Step s
             emits: AV[s-1] matmul, S[s] matmul (1024 cols), exp[s] on the
             scalar engine, one projection matmul for pair p+1.
  norm:      row sums land in rs_pack[128,128] via partition-0 scratch + DMA
             scatter; two [64,128] legacy reciprocals (after pair 3 / pair 7);
             DRAM-bounce partition broadcast; one in-place multiply per pair.
  out proj:  ch0 hp-outer (overlaps the tail normalization chain), ch1
             it-outer (streams output DMAs).
"""

import numpy as np

N = 1024
D = 1024
H = 16
DH = 64
P = 128
SCALE = float(D) ** (-0.5)
NCORES = 8

_STATE: dict = {}


def _emit(tc, xT, wqkv, wout, bout, out):
    import concourse.mybir as mybir

    nc = tc.nc
    f32 = mybir.dt.float32
    bf16 = mybir.dt.bfloat16
    EXP = mybir.ActivationFunctionType.Exp

    from contextlib import ExitStack

    with ExitStack() as ctx:
        persist = ctx.enter_context(tc.tile_pool(name="persist", bufs=1))
        v_aug = [persist.tile([P, H * 65], bf16, tag=f"vaug{nt}", name=f"vaug{nt}")
                 for nt in range(8)]
        oT = [persist.tile([P, N], bf16, tag=f"oT{hp}", name=f"oT{hp}") for hp in range(8)]
        bias_sb = persist.tile([P, N], f32, tag="bias", name="bias_sb")
        rs_scr = [persist.tile([1, N], f32, tag=f"rss{i}", name=f"rs_scr{i}")
                  for i in range(2)]
        # per-pair packed rowsums (16 partitions: head-even rows 0-7, odd 8-15)
        rs_pack = [persist.tile([H, P], f32, tag=f"rsp{i}", name=f"rs_pack{i}")
                   for i in range(2)]
        rcp_pack = [persist.tile([H, P], f32, tag=f"rcpp{i}", name=f"rcp_pack{i}")
                    for i in range(2)]
        ones_f = persist.tile([P, H], f32, tag="ones", name="ones_f")

        nc.vector.memset(ones_f[:], 1.0)

        with tc.tile_pool(name="xt", bufs=1) as xt_pool, \
             tc.tile_pool(name="wv", bufs=1) as wv_pool, \
             tc.tile_pool(name="wstream", bufs=2) as wpool, \
             tc.tile_pool(name="wo", bufs=2) as wo_pool, \
             tc.tile_pool(name="qk", bufs=2) as qk_pool, \
             tc.tile_pool(name="p", bufs=1) as p_pool, \
             tc.tile_pool(name="bc", bufs=2) as bc_pool, \
             tc.tile_pool(name="osb", bufs=2) as o_pool, \
             tc.tile_pool(name="dbounce", bufs=1, space="DRAM") as d_pool, \
             tc.tile_pool(name="ps", bufs=1, space="PSUM") as sp:

            dram_t2 = d_pool.tile([H, N], f32, tag="d2", name="dram_t2")

            # ---------------- input DMAs ----------------
            # round-robin (wv[k], xT[k]) across the three DMA queues so
            # arrival order tracks k; bias/wq/wk go behind them
            q3 = [nc.sync, nc.scalar, nc.gpsimd]
            xT_sb, wv_sb = [], []
            for k in range(8):
                w = wv_pool.tile([P, N], bf16, tag=f"wv{k}", name=f"wv{k}")
                q3[(2 * k) % 3].dma_start(w[:], wqkv[k * P:(k + 1) * P, 2048:3072])
                wv_sb.append(w)
                t = xt_pool.tile([P, N], bf16, tag=f"xt{k}", name=f"xt{k}")
                q3[(2 * k + 1) % 3].dma_start(t[:], xT[k * P:(k + 1) * P, :])
                xT_sb.append(t)

            def load_w(lo, eng):
                tiles = []
                for k in range(8):
                    w = wpool.tile([P, 512], bf16, tag=f"w{k}", name=f"w{k}")
                    eng.dma_start(w[:], wqkv[k * P:(k + 1) * P, lo:lo + 512])
                    tiles.append(w)
                return tiles

            # weights behind the inputs in queue order (needed later)
            wq_c = load_w(0, nc.sync)
            wk_c = load_w(1024, nc.gpsimd)
            nc.scalar.dma_start(bias_sb[:], bout[0:1, :].broadcast_to([P, N]))

            # ones column of v_aug
            for nt in range(8):
                nc.vector.tensor_copy(
                    v_aug[nt][:].rearrange("p (h e) -> p h e", e=65)[:, :, 64:65],
                    ones_f[:, :, None])

            def alloc_qk():
                q = qk_pool.tile([P, N], bf16, tag="q", name="qTt")
                k = qk_pool.tile([P, N], bf16, tag="k", name="kTt")
                return q, k

            def proj_mm(ps_t, wt, off, k):
                # one k-step of a projection chunk: both 512-col halves
                for ic in range(2):
                    nc.tensor.matmul(
                        ps_t[:, ic * 512:(ic + 1) * 512],
                        lhsT=wt[k][:, off:off + P],
                        rhs=xT_sb[k][:, ic * 512:(ic + 1) * 512],
                        start=(k == 0), stop=(k == 7))

            def v_mm(nt, ps_t, k):
                for ic in range(2):
                    nc.tensor.matmul(
                        ps_t[:, ic * 512:(ic + 1) * 512],
                        lhsT=xT_sb[k][:, nt * P:(nt + 1) * P],
                        rhs=wv_sb[k][:, ic * 512:(ic + 1) * 512],
                        start=(k == 0), stop=(k == 7))

            def v_copy(nt, ps_t):
                nc.vector.tensor_copy(
                    v_aug[nt][:].rearrange("p (h e) -> p h e", e=65)[:, :, 0:64],
                    ps_t[:].rearrange("p (h e) -> p h e", e=64))

            # ---------------- V projection + pair-0 prologue ----------------
            # group 1: V chunks 0-3 wave-major over 4 psum accumulators (as
            # (xT[k], wv[k]) arrive, all in-flight chunks advance one k-step);
            # group 2: V chunks 4-6 with the pair-0 q projection on slot 4;
            # group 3: V chunk 7 with the pair-0 k projection. Prologue copies
            # run on the idle scalar engine, keeping the DVE queue clear.
            qk_cur = alloc_qk()
            vps = [sp.tile([P, N], f32, tag=t, name="v_ps")
                   for t in ("sps0", "sps1", "ot", "proj")]
            for k in range(8):
                for c in range(4):
                    v_mm(c, vps[c], k)
            for c in range(4):
                v_copy(c, vps[c])

            vps2 = [sp.tile([P, N], f32, tag=t, name="v_ps2")
                    for t in ("sps0", "sps1", "ot")]
            q_ps = sp.tile([P, N], f32, tag="proj", name="q_ps")
            for k in range(8):
                for c in range(3):
                    v_mm(4 + c, vps2[c], k)
                proj_mm(q_ps, wq_c, 0, k)
            for c in range(3):
                v_copy(4 + c, vps2[c])
            nc.scalar.copy(qk_cur[0][:], q_ps[:])

            v7_ps = sp.tile([P, N], f32, tag="sps0", name="v7_ps")
            k_ps = sp.tile([P, N], f32, tag="sps1", name="k_ps")
            for k in range(8):
                v_mm(7, v7_ps, k)
                proj_mm(k_ps, wk_c, 0, k)
            v_copy(7, v7_ps)
            nc.scalar.copy(qk_cur[1][:], k_ps[:])

            # ---------------- attention ----------------
            pend = None

            def emit_av(h, jt, pt, ot_t):
                va = v_aug[jt][:].rearrange("p (h e) -> p h e", e=65)[:, h, :]
                for ic in range(2):
                    nc.tensor.matmul(ot_t[0:65, ic * 512:(ic + 1) * 512], lhsT=va,
                                     rhs=pt[:, ic * 512:(ic + 1) * 512],
                                     start=(jt == 0), stop=(jt == 7))

            def emit_stage(h, ot_t, rs_eng=None):
                # rs_eng: engine for the rowsum-row copy (scalar engine at the
                # tail, where it is idle and off the DVE critical chain)
                pr, hf = divmod(h, 2)
                sc = rs_scr[h % 2]
                if rs_eng is nc.scalar:
                    nc.scalar.copy(sc[0:1, :], ot_t[64:65, :])
                else:
                    nc.vector.tensor_copy(sc[0:1, :], ot_t[64:65, :])
                nc.vector.tensor_copy(oT[pr][64 * hf:64 * hf + 64, :], ot_t[0:64, :])
                nc.gpsimd.dma_start(rs_pack[pr % 2][8 * hf:8 * hf + 8, :], sc[0:1, :])

            def emit_norm_pair(p, last=False):
                # reciprocal of this pair's packed row sums, DRAM bounce,
                # partition-broadcast, in-place scale of oT[p]. The tail
                # pair's multiply runs on gpsimd: all DVE completions share a
                # semaphore counter, and the wait-merge pass would otherwise
                # make every out-projection matmul wait for this last multiply.
                rp = rs_pack[p % 2]
                cp = rcp_pack[p % 2]
                nc.vector.reciprocal(cp[0:H, :], rp[0:H, :])
                nc.sync.dma_start(
                    dram_t2[2 * p:2 * p + 2, :].rearrange("a (b c) -> (a b) c", b=8),
                    cp[0:H, :])
                bc = bc_pool.tile([P, N], f32, tag=f"bc{p % 2}", name="bc")
                nc.sync.dma_start(
                    bc[0:64, :], dram_t2[2 * p:2 * p + 1, :].broadcast_to([64, N]))
                nc.gpsimd.dma_start(
                    bc[64:128, :],
                    dram_t2[2 * p + 1:2 * p + 2, :].broadcast_to([64, N]))
                nc.vector.tensor_mul(oT[p][:], oT[p][:], bc[:])

            PSCHED = ([(0, [k]) for k in range(8)] +
                      [(1, [0, 1]), (1, [2, 3]), (1, [4]), (1, [5]), (1, [6]),
                       (1, [7]), None, None])

            def load_wo(ch, eng):
                tiles = []
                for hp in range(8):
                    w = wo_pool.tile([P, 512], bf16, tag=f"wo{hp}", name=f"wo{hp}")
                    eng.dma_start(w[:], wout[hp * P:(hp + 1) * P, ch * 512:(ch + 1) * 512])
                    tiles.append(w)
                return tiles

            # two-step AV pipeline: pendq holds (head, jt, p_tile); ot psum is
            # allocated lazily when a head's first AV is emitted, after the
            # previous head's stage copies (so the ot-slot recycling dep holds)
            pendq = []
            ot_cur = [None]

            def drain_av(last=False):
                ph, pjt, ppt = pendq.pop(0)
                if pjt == 0:
                    ot_cur[0] = sp.tile([P, N], f32, tag="ot", name="ot_ps")
                emit_av(ph, pjt, ppt, ot_cur[0])
                if pjt == 7:
                    emit_stage(ph, ot_cur[0],
                               rs_eng=nc.scalar if last else None)

            sidx = 0
            for p in range(8):
                qT_c, kT_c = qk_cur
                if p == 3:
                    wq_c1 = load_w(512, nc.sync)
                    wk_c1 = load_w(1024 + 512, nc.gpsimd)
                if p == 6:
                    # prefetch output-projection weights so they are resident
                    # before the sync queue fills with tail bc broadcasts
                    wo0 = load_wo(0, nc.sync)
                    wo1 = load_wo(1, nc.gpsimd)
                if p < 7:
                    qk_next = alloc_qk()
                    wq_n = wq_c if (p + 1) < 4 else wq_c1
                    wk_n = wk_c if (p + 1) < 4 else wk_c1
                    off_n = ((p + 1) % 4) * P
                for s in range(16):
                    hf, jt = s // 8, s % 8
                    h = 2 * p + hf
                    bp = 64 * hf
                    if len(pendq) >= 2:
                        drain_av()
                    if p > 0 and s == 3:
                        emit_norm_pair(p - 1)
                    sps = sp.tile([P, N], f32, tag=f"sps{sidx % 2}", name="s_ps")
                    for ic in range(2):
                        nc.tensor.matmul(
                            sps[:, ic * 512:(ic + 1) * 512],
                            lhsT=kT_c[bp:bp + 64, jt * P:(jt + 1) * P],
                            rhs=qT_c[bp:bp + 64, ic * 512:(ic + 1) * 512],
                            start=True, stop=True)
                    pt = p_pool.tile([P, N], bf16, tag=f"p{sidx % 5}", name="p_sb")
                    nc.scalar.activation(pt[:], sps[:], EXP, scale=SCALE)
                    # projection k-steps for pair p+1; the k chunk finishes by
                    # step 13 so its copy lands before the next pair's S needs it
                    if p < 7 and PSCHED[s] is not None:
                        c, kks = PSCHED[s]
                        if kks[0] == 0:
                            proj_ps = sp.tile([P, N], f32, tag="proj", name="proj_ps")
                        wt = wq_n if c == 0 else wk_n
                        for kk in kks:
                            proj_mm(proj_ps, wt, off_n, kk)
                        if kks[-1] == 7:
                            nc.vector.tensor_copy(qk_next[c][:], proj_ps[:])
                    pendq.append((h, jt, pt))
                    sidx += 1
                if p < 7:
                    qk_cur = qk_next
            drain_av()
            drain_av(last=True)

            # ---------------- output projection ----------------
            def finish_tile(it, ch, ps_t, eng):
                osb = o_pool.tile([P, 512], f32, tag=f"o{it % 2}", name="o_sb")
                nc.vector.tensor_add(osb[:], ps_t,
                                     bias_sb[0:P, ch * 512:(ch + 1) * 512])
                eng.dma_start(out[it * P:(it + 1) * P, ch * 512:(ch + 1) * 512], osb[:])

            # ch0 hp-outer. Levels 0-6 are emitted BEFORE the pair-7
            # normalization so they cannot inherit its semaphore wait (the
            # wait-merge pass would otherwise stall the whole out-projection
            # on the tail chain and drop the PE p-state). The ot-tag slots sit
            # at it=6/7, giving their stage-15 WAR time to clear.
            # slot order = availability order: proj's last reader is ancient,
            # sps0/sps1 wait the final exps, ot waits the stage-15 copy; ch1's
            # recycle rotation below consumes finishes in this same order
            fslots = []
            for tg in ("proj", "sps0", "sps1", "ot"):
                t = sp.tile([P, N], f32, tag=tg, name="f_ps")
                fslots.append(t[:, 0:512])
                fslots.append(t[:, 512:1024])
            for hp in range(7):
                for it in range(8):
                    nc.tensor.matmul(
                        fslots[it], lhsT=oT[hp][:, it * P:(it + 1) * P],
                        rhs=wo0[hp][:], start=(hp == 0), stop=False)

            # tail normalization for pair 7 (chain hides under ch0 levels 0-6)
            emit_norm_pair(7, last=True)

            for it in range(8):
                nc.tensor.matmul(
                    fslots[it], lhsT=oT[7][:, it * P:(it + 1) * P],
                    rhs=wo0[7][:], start=False, stop=True)
            for it in range(8):
                finish_tile(it, 0, fslots[it], nc.sync if it % 2 == 0 else nc.gpsimd)

            # ch1: it-outer, rotating over freed slots in finish order
            for it in range(8):
                ps_t = sp.tile([P, 512], f32, tag=["proj", "sps0", "sps1"][it % 3],
                               name="f1_ps")
                for hp in range(8):
                    nc.tensor.matmul(
                        ps_t[:], lhsT=oT[hp][:, it * P:(it + 1) * P],
                        rhs=wo1[hp][:], start=(hp == 0), stop=(hp == 7))
                finish_tile(it, 1, ps_t[:], nc.sync)


def build(mm_dtype: str = "bfloat16"):
    key = ("nc", mm_dtype)
    if key in _STATE:
        return _STATE[key]
    import concourse.mybir as mybir
    import concourse.tile as tile
    from concourse import bacc

    nc = bacc.Bacc("TRN2", target_bir_lowering=False, debug=False,
                   enable_asserts=False, num_devices=NCORES)
    f32 = mybir.dt.float32
    bf16 = mybir.dt.bfloat16
    xT = nc.dram_tensor("xT", [D, N], bf16, kind="ExternalInput").ap()
    wqkv = nc.dram_tensor("wqkv", [D, 3 * D], bf16, kind="ExternalInput").ap()
    wout = nc.dram_tensor("wout", [D, D], bf16, kind="ExternalInput").ap()
    bout = nc.dram_tensor("bout", [1, D], f32, kind="ExternalInput").ap()
    out = nc.dram_tensor("out", [N, D], f32, kind="ExternalOutput").ap()

    with tile.TileContext(nc) as tc:
        _emit(tc, xT, wqkv, wout, bout, out)
    nc.compile()
    _STATE[key] = nc
    return nc


def make_in_maps(x, w_qkv, w_out, b_out):
    import ml_dtypes
    bf = ml_dtypes.bfloat16
    x = np.asarray(x, np.float32)
    w_qkv = np.ascontiguousarray(np.asarray(w_qkv, np.float32)).astype(bf)
    w_out = np.ascontiguousarray(np.asarray(w_out, np.float32)).astype(bf)
    b_out = np.ascontiguousarray(np.asarray(b_out, np.float32)).reshape(1, D)
    return [
        {"xT": np.ascontiguousarray(x[b].T).astype(bf), "wqkv": w_qkv,
         "wout": w_out, "bout": b_out}
        for b in range(x.shape[0])
    ]


def run(x, w_qkv, w_out, b_out, trace=False, mm_dtype="bfloat16"):
    from concourse.bass_utils import run_bass_kernel_spmd

    nc = build(mm_dtype)
    in_maps = make_in_maps(x, w_qkv, w_out, b_out)
    res = run_bass_kernel_spmd(nc, in_maps, core_ids=list(range(NCORES)), trace=trace)
    outs = np.stack([res.results[c]["out"] for c in range(NCORES)])
    return outs, res


def kernel(x, w_qkv, w_out, b_out):
    outs, _ = run(x, w_qkv, w_out, b_out, trace=False)
    return outs.astype(np.float32)

